# revision 1
# baseline (speedup 1.0000x reference)
"""Trainium2 Bass kernel for the CustomGRU cell.

Reference computation (B=262144, D=128, fp32):
    b_z = colsum(B_update); b_r = colsum(B_reset); b_h = colsum(B_h)
    z      = sigmoid(x @ W_update + h @ U_update + b_z)
    r      = sigmoid(x @ W_reset  + x @ U_reset  + b_r)
    h_cand = tanh   (x @ W_h.T    + (r*h) @ U_h.T + b_h)
    h_t    = (1-z)*h + z*h_cand
    return h_t, h_cand

Strategy: pure data parallelism over the batch across 8 NeuronCores.
On the host we pre-transpose x/h shards to feature-major [128, B/8]
(so the contraction dim lands on SBUF partitions and every DMA is
contiguous), pre-combine W_reset+U_reset (both multiply x), pre-
transpose W_h/U_h, and pre-reduce the three bias matrices to vectors.
The device kernel is a streaming loop: 1 MiB-granularity DMAs, five
128x128 matmuls per 512-column subtile on TensorE, the three
nonlinearities (fused with the +bias) on ScalarE, and the gate blend
on VectorE/GpSimd. All data is 4-byte fp32; matmul operands are
declared float32r so the PE streams them at full (bf16) rate — HW
rounds matmul inputs to ~tf32 precision, giving ~3e-4 rel error while
the kernel stays DMA-bound at the ~457 GB/s/core streaming floor
(measured ~150 us/core for the 64 MiB of HBM traffic).
"""

import numpy as np

import concourse.bacc as bacc
import concourse.mybir as mybir
import concourse.tile as tile
from concourse.bass_utils import run_bass_kernel_spmd

N_CORES = 8
B_FULL = 262144
D = 128
B_LOC = B_FULL // N_CORES  # 32768 rows per core

F32 = mybir.dt.float32
F32R = mybir.dt.float32r
AF = mybir.ActivationFunctionType


def build_gru(nc, b_loc, chunk=2048, sub=512, nrep=1, io_bufs=3, mid_bufs=3, ps_bufs=2):
    """Emit the per-core GRU program on `nc` (feature-major layout).

    Inputs  : xt, ht      [D, b_loc]   (x/h shard, transposed)
              w_all       [5*D, D]     rows: W_z | U_z | W_r+U_r | W_h.T | U_h.T
              b_all       [D, 3]       cols: b_z | b_r | b_h
    Outputs : ht_out, hc_out [D, b_loc]  (h_t / h_cand shard, transposed)
    """
    xt = nc.dram_tensor("xt", [D, b_loc], F32R, kind="ExternalInput").ap()
    ht = nc.dram_tensor("ht", [D, b_loc], F32R, kind="ExternalInput").ap()
    wa = nc.dram_tensor("w_all", [5 * D, D], F32R, kind="ExternalInput").ap()
    ba = nc.dram_tensor("b_all", [D, 3], F32, kind="ExternalInput").ap()
    hto = nc.dram_tensor("ht_out", [D, b_loc], F32, kind="ExternalOutput").ap()
    hco = nc.dram_tensor("hc_out", [D, b_loc], F32, kind="ExternalOutput").ap()

    n_chunks = b_loc // chunk
    n_sub = chunk // sub

    with tile.TileContext(nc) as tc:
        with (
            tc.tile_pool(name="w", bufs=1) as wpool,
            tc.tile_pool(name="io", bufs=io_bufs) as io,
            tc.tile_pool(name="mid", bufs=mid_bufs) as mid,
            tc.tile_pool(name="ps", bufs=ps_bufs, space="PSUM") as ps,
        ):
            # First chunk's x/h loads are emitted before the weight DMAs so
            # the bulk stream starts immediately; weights ride behind them.
            first_x = io.tile([D, chunk], F32R, tag="x")
            nc.sync.dma_start(first_x[:], xt[:, 0:chunk])
            first_h = io.tile([D, chunk], F32R, tag="h")
            nc.sync.dma_start(first_h[:], ht[:, 0:chunk])
            w = []
            for k in range(5):
                t = wpool.tile([D, D], F32R, tag=f"w{k}")
                nc.sync.dma_start(t[:], wa[k * D:(k + 1) * D, :])
                w.append(t[:])
            bt = wpool.tile([D, 3], F32, tag="b")
            nc.sync.dma_start(bt[:], ba[:, :])

            for rep in range(nrep):
                for ci in range(n_chunks):
                    lo = ci * chunk
                    hi = lo + chunk
                    if ci == 0 and rep == 0:
                        xs, hs = first_x, first_h
                    else:
                        xs = io.tile([D, chunk], F32R, tag="x")
                        nc.sync.dma_start(xs[:], xt[:, lo:hi])
                        hs = io.tile([D, chunk], F32R, tag="h")
                        nc.sync.dma_start(hs[:], ht[:, lo:hi])
                    hsf = hs[:].bitcast(F32)
                    hts = io.tile([D, chunk], F32, tag="hto")
                    hcs = io.tile([D, chunk], F32, tag="hco")
                    for si in range(n_sub):
                        sl = slice(si * sub, (si + 1) * sub)
                        x_s = xs[:, sl]
                        h_s = hs[:, sl]

                        pz = ps.tile([D, sub], F32, tag="pz")
                        nc.tensor.matmul(pz[:], w[0], x_s, start=True, stop=False)
                        nc.tensor.matmul(pz[:], w[1], h_s, start=False, stop=True)
                        pr = ps.tile([D, sub], F32, tag="pr")
                        nc.tensor.matmul(pr[:], w[2], x_s, start=True, stop=True)

                        z_s = mid.tile([D, sub], F32, tag="z")
                        nc.scalar.activation(z_s[:], pz[:], AF.Sigmoid, bias=bt[:, 0:1])
                        r_s = mid.tile([D, sub], F32, tag="r")
                        nc.scalar.activation(r_s[:], pr[:], AF.Sigmoid, bias=bt[:, 1:2])

                        rh_s = mid.tile([D, sub], F32R, tag="rh")
                        nc.gpsimd.tensor_mul(rh_s[:], r_s[:], hsf[:, sl])

                        ph = ps.tile([D, sub], F32, tag="ph")
                        nc.tensor.matmul(ph[:], w[3], x_s, start=True, stop=False)
                        nc.tensor.matmul(ph[:], w[4], rh_s[:], start=False, stop=True)
                        nc.scalar.activation(hcs[:, sl], ph[:], AF.Tanh, bias=bt[:, 2:3])

                        d_s = mid.tile([D, sub], F32, tag="d")
                        nc.vector.tensor_sub(d_s[:], hcs[:, sl], hsf[:, sl])
                        zd_s = mid.tile([D, sub], F32, tag="zd")
                        nc.vector.tensor_mul(zd_s[:], z_s[:], d_s[:])
                        nc.vector.tensor_add(hts[:, sl], hsf[:, sl], zd_s[:])
                    if ci == n_chunks - 1 and rep == nrep - 1:
                        # Tail: store the final chunk in quarters, h_cand first
                        # (it is ready before h_t), so the last store drains as
                        # soon as the last blend lands instead of waiting for
                        # the whole 1 MiB chunk.
                        q4 = chunk // 4
                        for si2 in range(4):
                            s2 = slice(si2 * q4, (si2 + 1) * q4)
                            g2 = slice(lo + si2 * q4, lo + (si2 + 1) * q4)
                            nc.scalar.dma_start(hco[:, g2], hcs[:, s2])
                            nc.scalar.dma_start(hto[:, g2], hts[:, s2])
                    else:
                        nc.scalar.dma_start(hto[:, lo:hi], hts[:])
                        nc.scalar.dma_start(hco[:, lo:hi], hcs[:])
    return nc


def make_nc(b_loc=B_LOC, chunk=2048, sub=512, nrep=1):
    nc = bacc.Bacc(
        "TRN2",
        target_bir_lowering=False,
        debug=False,
        enable_asserts=False,
        num_devices=N_CORES,
    )
    build_gru(nc, b_loc, chunk=chunk, sub=sub, nrep=nrep)
    nc.compile()
    return nc


def host_prep(x, h, W_update, U_update, B_update, W_reset, U_reset, B_reset, W_h, U_h, B_h):
    """Host-side preprocessing: weight packing + per-core feature-major shards."""
    w_all = np.concatenate(
        [
            np.asarray(W_update, np.float32),
            np.asarray(U_update, np.float32),
            np.asarray(W_reset, np.float32) + np.asarray(U_reset, np.float32),
            np.asarray(W_h, np.float32).T,
            np.asarray(U_h, np.float32).T,
        ],
        axis=0,
    )
    w_all = np.ascontiguousarray(w_all, np.float32)
    b_all = np.stack(
        [
            np.asarray(B_update, np.float32).sum(axis=0),
            np.asarray(B_reset, np.float32).sum(axis=0),
            np.asarray(B_h, np.float32).sum(axis=0),
        ],
        axis=1,
    ).astype(np.float32)

    in_maps = []
    for c in range(N_CORES):
        rows = slice(c * B_LOC, (c + 1) * B_LOC)
        in_maps.append(
            {
                "xt": np.ascontiguousarray(np.asarray(x, np.float32)[rows].T),
                "ht": np.ascontiguousarray(np.asarray(h, np.float32)[rows].T),
                "w_all": w_all,
                "b_all": b_all,
            }
        )
    return in_maps


_NC_CACHE = {}


def kernel(**inputs):
    in_maps = host_prep(**inputs)
    if "nc" not in _NC_CACHE:
        _NC_CACHE["nc"] = make_nc()
    res = run_bass_kernel_spmd(_NC_CACHE["nc"], in_maps, list(range(N_CORES)))
    h_t = np.empty((B_FULL, D), np.float32)
    h_c = np.empty((B_FULL, D), np.float32)
    for c in range(N_CORES):
        rows = slice(c * B_LOC, (c + 1) * B_LOC)
        h_t[rows] = res.results[c]["ht_out"].T
        h_c[rows] = res.results[c]["hc_out"].T
    return h_t, h_c



# revision 2
# speedup vs baseline: 3.1431x; 3.1431x over previous
"""Trainium2 Bass kernel for the CustomGRU cell.

Reference computation (B=262144, D=128, fp32):
    b_z = colsum(B_update); b_r = colsum(B_reset); b_h = colsum(B_h)
    z      = sigmoid(x @ W_update + h @ U_update + b_z)
    r      = sigmoid(x @ W_reset  + x @ U_reset  + b_r)
    h_cand = tanh   (x @ W_h.T    + (r*h) @ U_h.T + b_h)
    h_t    = (1-z)*h + z*h_cand
    return h_t, h_cand

Strategy: data-parallel over batch across 8 cores, feature-major
[128, 32768] shards, bf16 end-to-end (inputs, weights, outputs).
bf16 halves HBM traffic vs fp32 (32 MiB/core streams in ~38 us) and
the loose 2e-2 tolerance leaves ample accuracy margin (~7e-3 measured).
Per 1024-column subtile: five 128x128xN bf16 matmuls (z: 2, r: 1,
cand: 2) into three PSUM accumulators; two sigmoids + tanh on ScalarE
(FD=1024 instructions, bias fused); r*h and the gate blend as four
bf16 tensor_tensor ops on VectorE (2x mode). PSUM: pz/pr single-
buffered (drained early by ACT), ph double-buffered = 8 banks.
"""

import numpy as np
import ml_dtypes

import concourse.bacc as bacc
import concourse.mybir as mybir
import concourse.tile as tile
from concourse.bass_utils import run_bass_kernel_spmd

N_CORES = 8
B_FULL = 262144
D = 128
B_LOC = B_FULL // N_CORES  # 32768 rows per core

F32 = mybir.dt.float32
BF16 = mybir.dt.bfloat16
AF = mybir.ActivationFunctionType


def build_gru(nc, b_loc=B_LOC, chunk=2048, sub=1024, nrep=1):
    """Per-core GRU program (feature-major layout, bf16).

    Inputs  : xt, ht      [D, b_loc] bf16  (x/h shard, transposed)
              w_all       [5*D, D]   bf16  rows: W_z | U_z | W_r+U_r | W_h.T | U_h.T
              b_all       [D, 3]     f32   cols: b_z | b_r | b_h
    Outputs : ht_out, hc_out [D, b_loc] bf16
    """
    xt = nc.dram_tensor("xt", [D, b_loc], BF16, kind="ExternalInput").ap()
    ht = nc.dram_tensor("ht", [D, b_loc], BF16, kind="ExternalInput").ap()
    wa = nc.dram_tensor("w_all", [5 * D, D], BF16, kind="ExternalInput").ap()
    ba = nc.dram_tensor("b_all", [D, 3], F32, kind="ExternalInput").ap()
    hto = nc.dram_tensor("ht_out", [D, b_loc], BF16, kind="ExternalOutput").ap()
    hco = nc.dram_tensor("hc_out", [D, b_loc], BF16, kind="ExternalOutput").ap()

    n_chunks = b_loc // chunk
    n_sub = chunk // sub
    nmm = sub // 512  # 512-wide matmuls per subtile

    with tile.TileContext(nc) as tc:
        with (
            tc.tile_pool(name="w", bufs=1) as wpool,
            tc.tile_pool(name="io", bufs=3) as io,
            tc.tile_pool(name="mid", bufs=3) as mid,
            tc.tile_pool(name="psA", bufs=1, space="PSUM") as psA,
            tc.tile_pool(name="psB", bufs=2, space="PSUM") as psB,
        ):
            # First chunk's x/h loads go out before the weights so the bulk
            # stream starts immediately.
            first_x = io.tile([D, chunk], BF16, tag="x")
            nc.sync.dma_start(first_x[:], xt[:, 0:chunk])
            first_h = io.tile([D, chunk], BF16, tag="h")
            nc.sync.dma_start(first_h[:], ht[:, 0:chunk])
            w = []
            for k in range(5):
                t = wpool.tile([D, D], BF16, tag=f"w{k}")
                nc.sync.dma_start(t[:], wa[k * D:(k + 1) * D, :])
                w.append(t[:])
            bt = wpool.tile([D, 3], F32, tag="b")
            nc.sync.dma_start(bt[:], ba[:, :])

            for rep in range(nrep):
                for ci in range(n_chunks):
                    lo = ci * chunk
                    hi = lo + chunk
                    if ci == 0 and rep == 0:
                        xs, hs = first_x, first_h
                    else:
                        xs = io.tile([D, chunk], BF16, tag="x")
                        nc.sync.dma_start(xs[:], xt[:, lo:hi])
                        hs = io.tile([D, chunk], BF16, tag="h")
                        nc.sync.dma_start(hs[:], ht[:, lo:hi])
                    hts = io.tile([D, chunk], BF16, tag="hto")
                    hcs = io.tile([D, chunk], BF16, tag="hco")
                    for si in range(n_sub):
                        sl = slice(si * sub, (si + 1) * sub)
                        x_s = xs[:, sl]
                        h_s = hs[:, sl]

                        pz = psA.tile([D, sub], F32, tag="pz")
                        pr = psA.tile([D, sub], F32, tag="pr")
                        for mi in range(nmm):
                            ml = slice(mi * 512, (mi + 1) * 512)
                            xm = xs[:, si * sub + mi * 512:si * sub + (mi + 1) * 512]
                            hm = hs[:, si * sub + mi * 512:si * sub + (mi + 1) * 512]
                            nc.tensor.matmul(pz[:, ml], w[0], xm, start=True, stop=False)
                            nc.tensor.matmul(pz[:, ml], w[1], hm, start=False, stop=True)
                            nc.tensor.matmul(pr[:, ml], w[2], xm, start=True, stop=True)

                        z_s = mid.tile([D, sub], BF16, tag="z")
                        nc.scalar.activation(z_s[:], pz[:], AF.Sigmoid, bias=bt[:, 0:1])
                        r_s = mid.tile([D, sub], BF16, tag="r")
                        nc.scalar.activation(r_s[:], pr[:], AF.Sigmoid, bias=bt[:, 1:2])

                        rh_s = mid.tile([D, sub], BF16, tag="rh")
                        nc.vector.tensor_mul(rh_s[:], r_s[:], h_s)

                        ph = psB.tile([D, sub], F32, tag="ph")
                        for mi in range(nmm):
                            ml = slice(mi * 512, (mi + 1) * 512)
                            xm = xs[:, si * sub + mi * 512:si * sub + (mi + 1) * 512]
                            nc.tensor.matmul(ph[:, ml], w[3], xm, start=True, stop=False)
                            nc.tensor.matmul(ph[:, ml], w[4], rh_s[:, ml], start=False, stop=True)
                        nc.scalar.activation(hcs[:, sl], ph[:], AF.Tanh, bias=bt[:, 2:3])

                        d_s = mid.tile([D, sub], BF16, tag="d")
                        nc.vector.tensor_sub(d_s[:], hcs[:, sl], h_s)
                        m_s = mid.tile([D, sub], BF16, tag="m")
                        nc.vector.tensor_mul(m_s[:], z_s[:], d_s[:])
                        nc.vector.tensor_add(hts[:, sl], h_s, m_s[:])
                    if ci == n_chunks - 1 and rep == nrep - 1:
                        # Tail: store the final chunk in quarters, h_cand
                        # first, so the last store drains as soon as the last
                        # blend lands.
                        q4 = chunk // 4
                        for si2 in range(4):
                            s2 = slice(si2 * q4, (si2 + 1) * q4)
                            g2 = slice(lo + si2 * q4, lo + (si2 + 1) * q4)
                            nc.scalar.dma_start(hco[:, g2], hcs[:, s2])
                            nc.scalar.dma_start(hto[:, g2], hts[:, s2])
                    else:
                        nc.scalar.dma_start(hto[:, lo:hi], hts[:])
                        nc.scalar.dma_start(hco[:, lo:hi], hcs[:])
    return nc


def make_nc(b_loc=B_LOC, chunk=2048, sub=1024, nrep=1):
    nc = bacc.Bacc(
        "TRN2",
        target_bir_lowering=False,
        debug=False,
        enable_asserts=False,
        num_devices=N_CORES,
    )
    build_gru(nc, b_loc, chunk=chunk, sub=sub, nrep=nrep)
    nc.compile()
    return nc


def host_prep(x, h, W_update, U_update, B_update, W_reset, U_reset, B_reset, W_h, U_h, B_h):
    """Host-side preprocessing: weight packing + per-core feature-major bf16 shards."""
    w_all = np.concatenate(
        [
            np.asarray(W_update, np.float32),
            np.asarray(U_update, np.float32),
            np.asarray(W_reset, np.float32) + np.asarray(U_reset, np.float32),
            np.asarray(W_h, np.float32).T,
            np.asarray(U_h, np.float32).T,
        ],
        axis=0,
    ).astype(ml_dtypes.bfloat16)
    b_all = np.stack(
        [
            np.asarray(B_update, np.float32).sum(axis=0),
            np.asarray(B_reset, np.float32).sum(axis=0),
            np.asarray(B_h, np.float32).sum(axis=0),
        ],
        axis=1,
    ).astype(np.float32)

    xt_full = np.asarray(x, np.float32).T.astype(ml_dtypes.bfloat16)
    ht_full = np.asarray(h, np.float32).T.astype(ml_dtypes.bfloat16)
    in_maps = []
    for c in range(N_CORES):
        cols = slice(c * B_LOC, (c + 1) * B_LOC)
        in_maps.append(
            {
                "xt": np.ascontiguousarray(xt_full[:, cols]),
                "ht": np.ascontiguousarray(ht_full[:, cols]),
                "w_all": w_all,
                "b_all": b_all,
            }
        )
    return in_maps


_NC_CACHE = {}


def kernel(**inputs):
    in_maps = host_prep(**inputs)
    if "nc" not in _NC_CACHE:
        _NC_CACHE["nc"] = make_nc()
    res = run_bass_kernel_spmd(_NC_CACHE["nc"], in_maps, list(range(N_CORES)))
    h_t = np.empty((B_FULL, D), np.float32)
    h_c = np.empty((B_FULL, D), np.float32)
    for c in range(N_CORES):
        rows = slice(c * B_LOC, (c + 1) * B_LOC)
        h_t[rows] = res.results[c]["ht_out"].astype(np.float32).T
        h_c[rows] = res.results[c]["hc_out"].astype(np.float32).T
    return h_t, h_c
